# revision 16
# baseline (speedup 1.0000x reference)
"""Trainium2 Bass kernel for the conv->softmax->NLL loss (nn_ARM_71665824301873).

Math (per reference):
  h = Conv2d(1->256, 3x3, pad=1)(x) + b          # (N, 256, 64, 64)
  prob = softmax over classes; idx = floor(x*255)
  loss = mean_n [ sum_pix ( LSE(h) - h[idx] ) ]   # clamp in reference never
                                                  # fires for this regime
Strategy: pure data parallel, 8 images per core (N=64 over 8 cores).
Per core:
  - conv as K=10 matmul (9 taps + ones-row for bias), classes on PSUM
    partitions, pixels streamed on the free dim.
  - exp on ScalarE (PSUM->SBUF bf16), class-sum via "indicator" matmuls
    (lhsT one-hot column -> per-image-quarter sums land on partitions 0..3),
    log+accumulate on ScalarE.
  - gathered term h[idx]: GPSIMD ap_gather pulls W[:, idx]/b[idx] from a
    256-entry table per pixel; fused multiply+reduce against the patch rows
    on VectorE (scalar_tensor_tensor accum_out).
  - host sums the 8 per-core partials and divides by N (data-parallel mean).
"""

import numpy as np
import ml_dtypes

BF16 = ml_dtypes.bfloat16

N_CORES = 8
IMGS = 8          # images per core
H = Wd = 64
D = H * Wd        # 4096 pixels per image
K = 256           # classes
PW = 66           # padded image width for tap shifts
PW2 = 68          # host padding (extra ring so every tap view is a full copy)
IMG_PAD = PW * PW  # 4356

# taps: (dy, dx) in conv output terms h[i,j] += W[t] * x[i+dy, j+dx]
TAPS = [(dy, dx) for dy in (-1, 0, 1) for dx in (-1, 0, 1)]  # t = 3*(dy+1)+(dx+1)

_COMPILED = {}


def _build_nc(idx_shift: float):
    from contextlib import ExitStack

    import concourse.bass as bass
    import concourse.bacc as bacc
    import concourse.tile as tile
    import concourse.mybir as mybir

    f32 = mybir.dt.float32
    bf16 = mybir.dt.bfloat16
    i16 = mybir.dt.int16
    AF = mybir.ActivationFunctionType
    ALU = mybir.AluOpType

    nc = bacc.Bacc(None)
    xpad_d = nc.declare_dram_parameter("xpad", [IMGS, PW2, PW2], bf16, isOutput=False)
    xw_d = nc.declare_dram_parameter("xw", [128, D // 16], f32, isOutput=False)
    wq_d = nc.declare_dram_parameter("wq", [128, 256], bf16, isOutput=False)
    wtab_d = nc.declare_dram_parameter("wtab", [128, 256], f32, isOutput=False)
    ones4_d = nc.declare_dram_parameter("ones4", [128, 16], bf16, isOutput=False)
    ones2_d = nc.declare_dram_parameter("ones2", [2, PW, PW], bf16, isOutput=False)
    out_d = nc.declare_dram_parameter("out", [1, 1], f32, isOutput=True)

    with tile.TileContext(nc) as tc, ExitStack() as ctx:
        pers = ctx.enter_context(tc.tile_pool(name="pers", bufs=1))
        expp = ctx.enter_context(tc.tile_pool(name="expp", bufs=4))
        hps = ctx.enter_context(tc.tile_pool(name="hps", bufs=2, space="PSUM"))
        seps = ctx.enter_context(tc.tile_pool(name="seps", bufs=2, space="PSUM"))

        # ---------------- persistent SBUF tiles ----------------
        patch = pers.tile([128, 2 * IMG_PAD], bf16)   # tap rows
        wqs = pers.tile([128, 256], bf16)             # conv lhsT replicas
        wtab = pers.tile([128, 256], f32)             # gather table
        ones4 = pers.tile([128, 16], bf16)            # indicator lhsT blocks
        xw = pers.tile([128, D // 16], f32)           # x wrapped for idx
        idx = pers.tile([128, D // 16], i16)
        wsel = pers.tile([128, D], f32)               # gathered table rows
        selprod = pers.tile([128, D], f32)            # stt elementwise scratch
        lse_s = pers.tile([128, 1024], bf16)          # log() main-out scratch
        acc_lse = pers.tile([128, IMGS], f32)
        acc_hsel = pers.tile([128, 1], f32)
        red = pers.tile([128, 2], f32)
        fin = pers.tile([128, 1], f32)
        onescol = pers.tile([128, 1], f32)

        # ---------------- loads + patch build ----------------
        # zero first: rows 10..15 of each 16-strip must be 0 so the
        # pair-wide gather-dot (26 partitions) sums no garbage.
        nc.vector.memset(patch[:, :], 0.0)
        nc.sync.dma_start(wqs[:, :], wq_d[:, :])
        nc.sync.dma_start(wtab[:, :], wtab_d[:, :])
        nc.sync.dma_start(ones4[:, :], ones4_d[:, :])
        nc.sync.dma_start(xw[:, :], xw_d[:, :])

        # patch rows for conv: strip 32*Q+t holds imgs (2Q, 2Q+1), t = tap.
        for q in range(4):
            for t, (dy, dx) in enumerate(TAPS):
                src = xpad_d[2 * q:2 * q + 2, 1 + dy:1 + dy + PW, 1 + dx:1 + dx + PW]
                dst = patch[32 * q + t:32 * q + t + 1, :].rearrange(
                    "p (n r c) -> p n r c", n=2, r=PW)
                nc.sync.dma_start(dst, src[None])
            dst = patch[32 * q + 9:32 * q + 10, :].rearrange(
                "p (n r c) -> p n r c", n=2, r=PW)
            nc.sync.dma_start(dst, ones2_d[None, :, :, :])
        # odd-image copies for the gather dot: strip 16g (g odd) <- img g
        for g in (1, 3, 5, 7):
            nc.sync.dma_start(
                patch[16 * g:16 * g + 10, 0:IMG_PAD],
                patch[32 * (g // 2):32 * (g // 2) + 10, IMG_PAD:2 * IMG_PAD],
            )

        nc.vector.memset(acc_hsel[:, :], 0.0)
        nc.vector.memset(acc_lse[:, :], 0.0)
        nc.vector.memset(onescol[:, :], 1.0)

        # ---------------- idx + gather + gathered-dot ----------------
        nc.vector.tensor_scalar(idx[:, :], xw[:, :], 255.0, idx_shift,
                                ALU.mult, ALU.add)
        nc.gpsimd.ap_gather(
            wsel[:, :].rearrange("p (n d) -> p n d", d=1),
            wtab[:, :].rearrange("p (n d) -> p n d", d=1),
            idx[:, :],
            channels=128, num_elems=256, d=1, num_idxs=D,
        )

        def img_views(g):
            # conv strip for img g
            q = g // 2
            off = (g % 2) * IMG_PAD
            conv_v = patch[32 * q:32 * q + 10, off:off + IMG_PAD].rearrange(
                "p (r c) -> p r c", c=PW)[:, 1:65, 1:65]
            return conv_v

        # ---------------- main pipeline over images ----------------
        for g in range(IMGS):
            q = g // 2
            conv_v = img_views(g)
            se = seps.tile([128, 1024], f32, tag="se")
            for s in range(4):           # four 1024-pixel quarters
                ehs = []
                for hh in range(2):      # class halves
                    hp = hps.tile([128, 1024], f32, tag="h")
                    for sub in range(2):  # 512-px (8-row) matmuls
                        r0 = s * 16 + sub * 8
                        nc.tensor.matmul(
                            hp[:, sub * 512:(sub + 1) * 512],
                            wqs[32 * q:32 * q + 10, hh * 128:(hh + 1) * 128],
                            conv_v[:, r0:r0 + 8, :],
                            start=True, stop=True,
                            tile_position=(32 * q, 0),
                        )
                    eh = expp.tile([128, 1024], bf16, tag="eh")
                    nc.scalar.activation(eh[:, :], hp[:, :], AF.Exp)
                    ehs.append(eh)
                # class-sum via indicator matmuls: quarter s -> partition s
                for u in range(2):       # 512-px chunks within the quarter
                    for hh in range(2):
                        nc.tensor.matmul(
                            se[0:4, u * 512:u * 512 + 512],
                            ones4[:, 4 * s:4 * s + 4],
                            ehs[hh][:, u * 512:(u + 1) * 512],
                            start=(s == 0 and hh == 0),
                            stop=(s == 3 and hh == 1),
                            tile_position=(0, 0),
                        )
            # se[s, j] = sumexp(img g, pixel s*1024 + j-chunk)
            nc.scalar.activation(lse_s[0:4, 0:1024], se[0:4, :], AF.Ln,
                                 accum_out=acc_lse[0:4, g:g + 1])
            # gathered-term dot, once per image pair (26-partition AP:
            # rows 0-9 even img, 10-15 zeros, 16-25 odd img copy)
            if g % 2 == 1:
                pv = patch[32 * q:32 * q + 26, 0:IMG_PAD].rearrange(
                    "p (r c) -> p r c", c=PW)[:, 1:65, 1:65]
                wv = wsel[32 * q:32 * q + 26, :].rearrange(
                    "p (r c) -> p r c", c=Wd)
                nc.vector.scalar_tensor_tensor(
                    selprod[32 * q:32 * q + 26, :].rearrange(
                        "p (r c) -> p r c", c=Wd),
                    pv, 1.0, wv,
                    ALU.mult, ALU.mult,
                    accum_out=acc_hsel[32 * q:32 * q + 26, 0:1],
                )

        # ---------------- final combine ----------------
        # partition-sums via tiny matmuls (ones-column lhsT), then subtract
        red_a = seps.tile([128, 1024], f32, tag="se")
        red_b = seps.tile([128, 1024], f32, tag="se")
        nc.tensor.matmul(red_a[0:1, 0:IMGS], onescol[0:4, 0:1],
                         acc_lse[0:4, :], start=True, stop=True,
                         tile_position=(0, 0))
        nc.tensor.matmul(red_b[0:1, 0:1], onescol[:, 0:1],
                         acc_hsel[:, :], start=True, stop=True,
                         tile_position=(0, 0))
        nc.vector.tensor_reduce(red[0:1, 0:1], red_a[0:1, 0:IMGS],
                                mybir.AxisListType.X, ALU.add)
        nc.vector.tensor_tensor(fin[0:1, 0:1], red[0:1, 0:1],
                                red_b[0:1, 0:1], ALU.subtract)
        nc.sync.dma_start(out_d[:, :], fin[0:1, 0:1])

    nc.finalize()
    return nc


def _host_inputs(x, W, b):
    """Per-core input maps (host-side re-layout only)."""
    x = np.ascontiguousarray(np.asarray(x, dtype=np.float32).reshape(64, H, Wd))
    W = np.asarray(W, dtype=np.float32).reshape(K, 3, 3)
    b = np.asarray(b, dtype=np.float32)

    # conv lhsT replicas: strip 32Q+t rows, col block hh -> W[t, class]
    wq = np.zeros((128, 256), dtype=BF16)
    wtab = np.zeros((128, 256), dtype=np.float32)
    for t, (dy, dx) in enumerate(TAPS):
        wrow = W[:, 1 + dy, 1 + dx]          # h[i,j] += W[k, 1+dy, 1+dx]*x[i+dy,j+dx]
        for q in range(4):
            wq[32 * q + t, :] = wrow.astype(BF16)
        for g in range(8):
            wtab[16 * g + t, :] = wrow
    for q in range(4):
        wq[32 * q + 9, :] = b.astype(BF16)
    for g in range(8):
        wtab[16 * g + 9, :] = b

    ones4 = np.zeros((128, 16), dtype=BF16)
    for qg in range(4):
        ones4[:, 4 * qg + qg] = BF16(1.0)

    ones2 = np.zeros((2, PW, PW), dtype=BF16)
    ones2[:, 1:65, 1:65] = BF16(1.0)

    in_maps = []
    for c in range(N_CORES):
        xs = x[c * IMGS:(c + 1) * IMGS]                      # (8, 64, 64) f32
        xpad = np.zeros((IMGS, PW2, PW2), dtype=BF16)
        xpad[:, 2:66, 2:66] = xs.astype(BF16)
        xw = np.ascontiguousarray(
            xs.reshape(IMGS, D // 16, 16).transpose(0, 2, 1).reshape(128, D // 16)
        ).astype(np.float32)
        in_maps.append({
            "xpad": np.ascontiguousarray(xpad),
            "xw": xw,
            "wq": wq,
            "wtab": wtab,
            "ones4": ones4,
            "ones2": ones2,
        })
    return in_maps


def kernel(x, W, b):
    from concourse.bass_utils import run_bass_kernel_spmd

    key = "main"
    if key not in _COMPILED:
        _COMPILED[key] = _build_nc(idx_shift=-0.5)
    nc = _COMPILED[key]
    in_maps = _host_inputs(x, W, b)
    res = run_bass_kernel_spmd(nc, in_maps, core_ids=list(range(N_CORES)))
    total = np.float64(0.0)
    for r in res.results:
        total += np.float64(r["out"].reshape(-1)[0])
    return np.float32(total / 64.0)
